# revision 18
# baseline (speedup 1.0000x reference)
"""Causal multi-head attention block (B=16, S=1024, d=1024, H=16) on 8 NeuronCores.

Strategy: data-parallel over batch (2 batches per core), no collectives.
Per-core kernel (fp16 matmuls, fp32 accumulation):
  phase A: x / W transposes via XBAR DMA-transpose (f32->f16 cast on ACT,
           then dma_start_transpose) -- keeps the PE free for matmuls
  phase B: QT = Wq @ xT, KT = Wk @ xT (transposed layout [d_out, m]),
           V  = x @ Wv.T (natural layout [m, d_out], packed in 65-wide
           per-head strips with a fused ones column)
  phase C: qc-major, head-PAIR iteration. The two heads of a pair have
           dh=64 contractions at partitions 0:64 / 64:128, so their score
           matmuls run CONCURRENTLY on disjoint PE row groups (auto
           tile_position from the lhsT base partition). exp((s+mask)/8) on
           ACT, causal mask via 0/1 triangle multiply on diagonal blocks,
           A@V accumulated per head (65-wide V strip with ones column ->
           sum row). Unnormalized outputs + sum rows evacuate on DVE;
           after every 4 pairs one batched DVE reciprocal_approx_fast
           computes 1/sums, chunked DMA broadcasts fan them out and one
           in-place DVE multiply per pair normalizes (ACT stays exp-only).
  phase D: y = attn_outT.T @ WoT (natural layout) -> DRAM; m-tiles are
           interleaved into attention emission so they hide in ACT-bound
           stretches.
Biases: bq/bk are zero by problem spec (ignored); bv/bo folded in exactly
on the host (y += bv @ Wo.T + bo).
"""

import numpy as np

_CACHE: dict = {}

S = 1024
D = 1024
H = 16
DH = 64
BPC = 2           # batches per core
M = BPC * S       # tokens per core
NCORES = 8


def _build_nc():
    import concourse.bass as bass  # noqa: F401
    import concourse.mybir as mybir
    import concourse.tile as tile
    from concourse import bacc
    from contextlib import ExitStack

    f32 = mybir.dt.float32
    f16 = mybir.dt.float16
    EXPF = mybir.ActivationFunctionType.Exp

    nc = bacc.Bacc("TRN2", target_bir_lowering=False, debug=False,
                   num_devices=NCORES)

    x_d = nc.dram_tensor("x", [M, D], f32, kind="ExternalInput")
    wq_d = nc.dram_tensor("Wq", [D, D], f32, kind="ExternalInput")
    wk_d = nc.dram_tensor("Wk", [D, D], f32, kind="ExternalInput")
    wv_d = nc.dram_tensor("Wv", [D, D], f32, kind="ExternalInput")
    wo_d = nc.dram_tensor("Wo", [D, D], f32, kind="ExternalInput")
    tri_d = nc.dram_tensor("tri01", [128, 128], f16, kind="ExternalInput")
    y_d = nc.dram_tensor("y", [M, D], f32, kind="ExternalOutput")

    NMT = M // 128        # 16 m-tiles
    NDT = D // 128        # 8 d-tiles
    NMC = M // 512        # 4 m-chunks
    NOC = D // 512        # 2 o-chunks

    with tile.TileContext(nc) as tc, ExitStack() as top:
        consts = top.enter_context(tc.tile_pool(name="consts", bufs=1))
        persist = top.enter_context(tc.tile_pool(name="persist", bufs=1))
        wrot = top.enter_context(tc.tile_pool(name="wrot", bufs=1))
        stage = top.enter_context(tc.tile_pool(name="stage", bufs=2))
        stage16 = top.enter_context(tc.tile_pool(name="stage16", bufs=2))
        ystage = top.enter_context(tc.tile_pool(name="ystage", bufs=2))
        expp = top.enter_context(tc.tile_pool(name="expp", bufs=12))
        tmpp = top.enter_context(tc.tile_pool(name="tmpp", bufs=3))
        srp = top.enter_context(tc.tile_pool(name="srp", bufs=2))
        bcp = top.enter_context(tc.tile_pool(name="bcp", bufs=3))
        grpp = top.enter_context(tc.tile_pool(name="grpp", bufs=3))
        # PSUM: 2 + 4 + 2 = 8 banks
        psAcc = top.enter_context(tc.tile_pool(name="psAcc", bufs=2, space="PSUM"))
        psS = top.enter_context(tc.tile_pool(name="psS", bufs=4, space="PSUM"))
        psO = top.enter_context(tc.tile_pool(name="psO", bufs=2, space="PSUM"))

        tri01 = consts.tile([128, 128], f16, tag="tri")
        nc.sync.dma_start(out=tri01, in_=tri_d[:, :])

        # persistent activations (fp16)
        QT = persist.tile([128, NDT, M], f16, tag="QT")    # [o, m] transposed
        KT = persist.tile([128, NDT, M], f16, tag="KT")
        V = persist.tile([128, NMT, H * 65], f16, tag="V")  # [m, head strips]

        def load_transposed(dst, dram, ncols):
            """dst[:, it, c*128:(c+1)*128] = dram[c*128:(c+1)*128, :].T
            dst: [128, NDT, ncols] fp16; dram: [ncols, D] fp32.
            f32->f16 cast on ACT, transpose on the DMA XBAR."""
            for rt in range(ncols // 128):
                st = stage.tile([128, D], f32, tag="stage")
                nc.sync.dma_start(out=st, in_=dram[rt * 128:(rt + 1) * 128, :])
                stf = stage16.tile([128, D], f16, tag="stage16")
                nc.scalar.copy(out=stf, in_=st)
                nc.sync.dma_start_transpose(
                    out=dst[:, :, rt * 128:(rt + 1) * 128], in_=stf)

        # ---------- phases A+B: projections ----------
        # xT and AO share one slot: xT's last reader is the V projection,
        # AO's first writer is the attention evacuation (WAR via slot reuse)
        xTp = top.enter_context(tc.tile_pool(name="xTp", bufs=1))
        xT = xTp.tile([128, NDT, M], f16, tag="xTAO")
        load_transposed(xT, x_d, M)

        # Q and K projections -> transposed layout (batch-0 m-chunks first)
        for w_dram, dst in ((wq_d, QT), (wk_d, KT)):
            WT = wrot.tile([128, NDT, D], f16, tag="WT")
            load_transposed(WT, w_dram, D)
            for mc in range(NMC):
                for ot in range(NDT):
                    pp = psAcc.tile([128, 512], f32, tag="psAcc")
                    for it in range(NDT):
                        nc.tensor.matmul(
                            pp,
                            WT[:, it, ot * 128:(ot + 1) * 128],
                            xT[:, it, mc * 512:(mc + 1) * 512],
                            start=(it == 0), stop=(it == NDT - 1))
                    nc.scalar.copy(
                        out=dst[:, ot, mc * 512:(mc + 1) * 512], in_=pp)

        # V projection -> natural layout in 65-wide head strips:
        # every head h: [V(64) | ones] at cols h*65..h*65+64
        WT = wrot.tile([128, NDT, D], f16, tag="WT")
        load_transposed(WT, wv_d, D)
        for mt in range(NMT):
            v2 = V[:, mt, :].rearrange("p (a c) -> p a c", c=65)
            nc.gpsimd.memset(v2[:, :, 64], 1.0)
            for oc in range(NOC):
                pp = psAcc.tile([128, 512], f32, tag="psAcc")
                for it in range(NDT):
                    nc.tensor.matmul(
                        pp,
                        xT[:, it, mt * 128:(mt + 1) * 128],
                        WT[:, it, oc * 512:(oc + 1) * 512],
                        start=(it == 0), stop=(it == NDT - 1))
                nc.scalar.copy(
                    out=v2[:, 8 * oc:8 * oc + 8, 0:64],
                    in_=pp.rearrange("p (a c) -> p a c", c=64))

        # attn out (transposed layout), reuses xT's SBUF slot
        AO = xTp.tile([128, NDT, M], f16, tag="xTAO")

        # ---------- phase C: attention (per batch) ----------
        def normalize_group(b, qc, hpg, sgrp):
            """Reciprocal + normalize for pairs 4*hpg..4*hpg+3 at this qc
            (their 8 sum rows, collected in sgrp, are complete).
            sgrp row: (hp%4)*2 + (0 even head | 1 odd head)."""
            rg32 = grpp.tile([8, 512], f32, tag="rg32")
            rg = grpp.tile([8, 512], f16, tag="rg")
            nc.vector.reciprocal_approx_fast(out=rg32, in_=sgrp)
            nc.vector.tensor_copy(out=rg, in_=rg32)
            q0 = b * S + qc * 512
            for lp in range(4):
                p = 4 * hpg + lp                 # head pair == dt block
                bc = bcp.tile([128, 512], f16, tag="bc")
                for loc, p0 in ((2 * lp, 0), (2 * lp + 1, 64)):
                    r1 = rg[loc:loc + 1, :]
                    for ch in range(4):  # 4 parallel column chunks
                        rc = r1[:, ch * 128:(ch + 1) * 128]
                        rsrc = bass.AP(
                            tensor=rc.tensor, offset=rc.offset,
                            ap=[list(rc.ap[0]), [0, 64]]
                            + [list(a) for a in rc.ap[1:]])
                        nc.sync.dma_start(
                            out=bc[p0:p0 + 64, ch * 128:(ch + 1) * 128],
                            in_=rsrc)
                nc.vector.tensor_mul(
                    out=AO[:, p, q0:q0 + 512],
                    in0=AO[:, p, q0:q0 + 512], in1=bc)

        def attention_batch(b, interleave=None):
            for qc in range(2):
                nkt = (qc + 1) * 4
                q0 = b * S + qc * 512
                for hp in range(NDT):            # head pair == dt block
                    if hp % 4 == 0:
                        sgrp = grpp.tile([8, 512], f32, tag="sgrp")
                    hA, hB = 2 * hp, 2 * hp + 1
                    ps_oA = psO.tile([128, 512], f32, tag="psO")
                    ps_oB = psO.tile([128, 512], f32, tag="psO")

                    exts = []

                    def emit_scores(kt):
                        k0 = kt * 128
                        off = max(0, k0 - qc * 512)
                        kg = b * S + k0
                        exab = []
                        for po in (0, 64):   # even/odd head: PE row groups
                            ps_s = psS.tile([128, 512], f32, tag="psS")
                            nc.tensor.matmul(
                                ps_s[:, off:512],
                                KT[po:po + 64, hp, kg:kg + 128],
                                QT[po:po + 64, hp, q0 + off:q0 + 512],
                                start=True, stop=True)
                            ex = expp.tile([128, 512], f16, tag="exp")
                            nc.scalar.activation(
                                out=ex[:, off:512], in_=ps_s[:, off:512],
                                func=EXPF, scale=0.125)
                            if k0 >= qc * 512:  # diagonal: 0/1 triangle
                                nc.vector.tensor_mul(
                                    ex[:, off:off + 128],
                                    ex[:, off:off + 128], tri01)
                            exab.append(ex)
                        exts.append((exab[0], exab[1], off, kt))

                    def emit_av(i):
                        exA, exB, off, kt = exts[i]
                        mtv = b * (S // 128) + kt
                        for ex, ps_o, h in ((exA, ps_oA, hA),
                                            (exB, ps_oB, hB)):
                            nc.tensor.matmul(
                                ps_o[0:65, off:512],
                                V[:, mtv, h * 65:h * 65 + 65],
                                ex[:, off:512],
                                start=(kt == 0), stop=(kt == nkt - 1))

                    # software pipeline: scores run 4 blocks ahead of AV
                    for kt in range(nkt):
                        emit_scores(kt)
                        if kt >= 4:
                            emit_av(kt - 4)
                    for i in range(max(0, nkt - 4), nkt):
                        emit_av(i)

                    # evacuate unnormalized output + sum rows (DVE)
                    locA = (hp % 4) * 2
                    for loc, ps_o, po in ((locA, ps_oA, 0),
                                          (locA + 1, ps_oB, 64)):
                        srow = srp.tile([65, 512], f32, tag="srow")
                        nc.vector.tensor_copy(out=srow[64:65, :],
                                              in_=ps_o[64:65, :])
                        nc.sync.dma_start(out=sgrp[loc:loc + 1, :],
                                          in_=srow[64:65, :])
                        if po == 0:
                            nc.vector.tensor_copy(
                                out=AO[0:64, hp, q0:q0 + 512],
                                in_=ps_o[0:64, :])
                        else:
                            tmp = tmpp.tile([64, 512], f16, tag="tmp")
                            nc.vector.tensor_copy(out=tmp, in_=ps_o[0:64, :])
                            nc.sync.dma_start(
                                out=AO[64:128, hp, q0:q0 + 512], in_=tmp)
                    if hp % 4 == 3:
                        normalize_group(b, qc, hp // 4, sgrp)
                    if interleave is not None:
                        interleave(qc, hp)

        def out_proj_tile(mt, WoT):
            ys = ystage.tile([128, D], f32, tag="ys")
            for oc in range(NOC):
                pp = psAcc.tile([128, 512], f32, tag="psAcc")
                for dt_ in range(NDT):
                    nc.tensor.matmul(
                        pp,
                        AO[:, dt_, mt * 128:(mt + 1) * 128],
                        WoT[:, dt_, oc * 512:(oc + 1) * 512],
                        start=(dt_ == 0), stop=(dt_ == NDT - 1))
                nc.scalar.copy(out=ys[:, oc * 512:(oc + 1) * 512], in_=pp)
            nc.sync.dma_start(out=y_d[mt * 128:(mt + 1) * 128, :], in_=ys)

        attention_batch(0)
        WoT = wrot.tile([128, NDT, D], f16, tag="WT")
        load_transposed(WoT, wo_d, D)

        # attention b1 with phase-D m-tiles interleaved (they hide in the
        # ACT-bound stretches of attention):
        #   b1 qc=0 pairs: batch-0 m-tiles 0..7
        #   b1 qc=1 pairs: batch-1 qc=0 m-tiles 8..11
        def _ilv(qc, hp):
            if qc == 0:
                out_proj_tile(hp, WoT)
            elif hp % 2 == 1:
                out_proj_tile(8 + hp // 2, WoT)
        attention_batch(1, interleave=_ilv)
        for mt in range(12, 16):
            out_proj_tile(mt, WoT)

    nc.compile()
    return nc


def _tri01():
    # tri01[dk, dq] = 1 where k <= q (allowed), else 0
    return np.triu(np.ones((128, 128), np.float16))


def _get_nc():
    if "nc" not in _CACHE:
        _CACHE["nc"] = _build_nc()
    return _CACHE["nc"]


def kernel(x, Wq, bq, Wk, bk, Wv, bv, Wo, bo):
    from concourse.bass_utils import run_bass_kernel_spmd

    x = np.ascontiguousarray(np.asarray(x, dtype=np.float32))
    B = x.shape[0]
    assert x.shape == (B, S, D) and B == NCORES * BPC
    Wq = np.ascontiguousarray(np.asarray(Wq, dtype=np.float32))
    Wk = np.ascontiguousarray(np.asarray(Wk, dtype=np.float32))
    Wv = np.ascontiguousarray(np.asarray(Wv, dtype=np.float32))
    Wo = np.ascontiguousarray(np.asarray(Wo, dtype=np.float32))

    nc = _get_nc()
    shards = x.reshape(NCORES, M, D)
    tri = _tri01()
    in_maps = [
        {"x": shards[c], "Wq": Wq, "Wk": Wk, "Wv": Wv, "Wo": Wo, "tri01": tri}
        for c in range(NCORES)
    ]
    res = run_bass_kernel_spmd(nc, in_maps, core_ids=list(range(NCORES)))
    y = np.stack([res.results[c]["y"] for c in range(NCORES)])
    y = y.reshape(B, S, D)

    # exact host-side fold of bv and bo (bq/bk are zero by problem spec)
    bias = (np.asarray(bv, np.float32) @ np.asarray(Wo, np.float32).T
            + np.asarray(bo, np.float32))
    if np.any(bias):
        y = y + bias
    return y.astype(np.float32)


# revision 22
# speedup vs baseline: 1.1712x; 1.1712x over previous
"""Causal multi-head attention block (B=16, S=1024, d=1024, H=16) on 8 NeuronCores.

Strategy: data-parallel over batch (2 batches per core), no collectives.
Per-core kernel (fp16 matmuls, fp32 accumulation):
  phase A: x / W transposes via XBAR DMA-transpose (f32->f16 cast on ACT,
           then dma_start_transpose) -- keeps the PE free for matmuls
  phase B: QT = Wq @ xT, KT = Wk @ xT (transposed layout [d_out, m]),
           V  = x @ Wv.T (natural layout [m, d_out], packed in 65-wide
           per-head strips with a fused ones column)
  phase C: qc-major, head-PAIR iteration. The two heads of a pair have
           dh=64 contractions at partitions 0:64 / 64:128, so their score
           matmuls run CONCURRENTLY on disjoint PE row groups (auto
           tile_position from the lhsT base partition). exp((s+mask)/8) on
           ACT, causal mask via 0/1 triangle multiply on diagonal blocks,
           A@V accumulated per head (65-wide V strip with ones column ->
           sum row). Unnormalized outputs + sum rows evacuate on DVE;
           after every 4 pairs one batched DVE reciprocal_approx_fast
           computes 1/sums, chunked DMA broadcasts fan them out and one
           in-place DVE multiply per pair normalizes (ACT stays exp-only).
  phase D: y = attn_outT.T @ WoT (natural layout) -> DRAM; m-tiles are
           interleaved into attention emission so they hide in ACT-bound
           stretches.
Biases: bq/bk are zero by problem spec (ignored); bv/bo folded in exactly
on the host (y += bv @ Wo.T + bo).
"""

import numpy as np

_CACHE: dict = {}

S = 1024
D = 1024
H = 16
DH = 64
BPC = 2           # batches per core
M = BPC * S       # tokens per core
NCORES = 8


def _build_nc():
    import concourse.bass as bass  # noqa: F401
    import concourse.mybir as mybir
    import concourse.tile as tile
    from concourse import bacc
    from concourse.masks import make_identity
    from contextlib import ExitStack

    f32 = mybir.dt.float32
    f16 = mybir.dt.float16
    EXPF = mybir.ActivationFunctionType.Exp

    nc = bacc.Bacc("TRN2", target_bir_lowering=False, debug=False,
                   num_devices=NCORES)

    x_d = nc.dram_tensor("x", [M, D], f32, kind="ExternalInput")
    wq_d = nc.dram_tensor("Wq", [D, D], f32, kind="ExternalInput")
    wk_d = nc.dram_tensor("Wk", [D, D], f32, kind="ExternalInput")
    wv_d = nc.dram_tensor("Wv", [D, D], f32, kind="ExternalInput")
    wo_d = nc.dram_tensor("Wo", [D, D], f32, kind="ExternalInput")
    tri_d = nc.dram_tensor("tri01", [128, 128], f16, kind="ExternalInput")
    y_d = nc.dram_tensor("y", [M, D], f32, kind="ExternalOutput")

    NMT = M // 128        # 16 m-tiles
    NDT = D // 128        # 8 d-tiles
    NMC = M // 512        # 4 m-chunks
    NOC = D // 512        # 2 o-chunks

    with tile.TileContext(nc) as tc, ExitStack() as top:
        consts = top.enter_context(tc.tile_pool(name="consts", bufs=1))
        persist = top.enter_context(tc.tile_pool(name="persist", bufs=1))
        wrot = top.enter_context(tc.tile_pool(name="wrot", bufs=1))
        stage = top.enter_context(tc.tile_pool(name="stage", bufs=2))
        ystage = top.enter_context(tc.tile_pool(name="ystage", bufs=2))
        expp = top.enter_context(tc.tile_pool(name="expp", bufs=12))
        tmpp = top.enter_context(tc.tile_pool(name="tmpp", bufs=3))
        srp = top.enter_context(tc.tile_pool(name="srp", bufs=2))
        bcp = top.enter_context(tc.tile_pool(name="bcp", bufs=3))
        grpp = top.enter_context(tc.tile_pool(name="grpp", bufs=3))
        # PSUM: 3 + 3 + 2 = 8 banks
        psAcc = top.enter_context(tc.tile_pool(name="psAcc", bufs=3, space="PSUM"))
        psS = top.enter_context(tc.tile_pool(name="psS", bufs=3, space="PSUM"))
        psO = top.enter_context(tc.tile_pool(name="psO", bufs=2, space="PSUM"))

        ident = consts.tile([128, 128], f32, tag="ident")
        make_identity(nc, ident)
        tri01 = consts.tile([128, 128], f16, tag="tri")
        nc.sync.dma_start(out=tri01, in_=tri_d[:, :])

        # persistent activations (fp16)
        QT = persist.tile([128, NDT, M], f16, tag="QT")    # [o, m] transposed
        KT = persist.tile([128, NDT, M], f16, tag="KT")
        V = persist.tile([128, NMT, H * 65], f16, tag="V")  # [m, head strips]

        def load_transposed(dst, dram, ncols):
            """dst[:, i_tile, c*128:(c+1)*128] = dram[c*128:(c+1)*128, :].T
            dst: [128, NDT, ncols] fp16; dram: [ncols, D] fp32."""
            for rt in range(ncols // 128):
                st = stage.tile([128, D], f32, tag="stage")
                nc.sync.dma_start(out=st, in_=dram[rt * 128:(rt + 1) * 128, :])
                for g in range(NDT // 4):
                    pt = psAcc.tile([128, 512], f32, tag="psAcc")
                    for c in range(4):
                        ib = g * 4 + c
                        nc.tensor.transpose(
                            pt[:, c * 128:(c + 1) * 128],
                            st[:, ib * 128:(ib + 1) * 128], ident)
                    nc.scalar.copy(
                        out=dst[:, g * 4:g * 4 + 4, rt * 128:(rt + 1) * 128],
                        in_=pt.rearrange("p (a b) -> p a b", b=128))

        # ---------- phases A+B: projections ----------
        # xT and AO share one slot: xT's last reader is the V projection,
        # AO's first writer is the attention evacuation (WAR via slot reuse)
        xTp = top.enter_context(tc.tile_pool(name="xTp", bufs=1))
        xT = xTp.tile([128, NDT, M], f16, tag="xTAO")
        load_transposed(xT, x_d, M)

        # Q and K projections -> transposed layout (batch-0 m-chunks first)
        for w_dram, dst in ((wq_d, QT), (wk_d, KT)):
            WT = wrot.tile([128, NDT, D], f16, tag="WT")
            load_transposed(WT, w_dram, D)
            for mc in range(NMC):
                for ot in range(NDT):
                    pp = psAcc.tile([128, 512], f32, tag="psAcc")
                    for it in range(NDT):
                        nc.tensor.matmul(
                            pp,
                            WT[:, it, ot * 128:(ot + 1) * 128],
                            xT[:, it, mc * 512:(mc + 1) * 512],
                            start=(it == 0), stop=(it == NDT - 1))
                    nc.scalar.copy(
                        out=dst[:, ot, mc * 512:(mc + 1) * 512], in_=pp)

        # V projection -> natural layout in 65-wide head strips:
        # every head h: [V(64) | ones] at cols h*65..h*65+64
        WT = wrot.tile([128, NDT, D], f16, tag="WT")
        load_transposed(WT, wv_d, D)
        for mt in range(NMT):
            v2 = V[:, mt, :].rearrange("p (a c) -> p a c", c=65)
            nc.gpsimd.memset(v2[:, :, 64], 1.0)
            for oc in range(NOC):
                pp = psAcc.tile([128, 512], f32, tag="psAcc")
                for it in range(NDT):
                    nc.tensor.matmul(
                        pp,
                        xT[:, it, mt * 128:(mt + 1) * 128],
                        WT[:, it, oc * 512:(oc + 1) * 512],
                        start=(it == 0), stop=(it == NDT - 1))
                nc.scalar.copy(
                    out=v2[:, 8 * oc:8 * oc + 8, 0:64],
                    in_=pp.rearrange("p (a c) -> p a c", c=64))

        # attn out (transposed layout), reuses xT's SBUF slot
        AO = xTp.tile([128, NDT, M], f16, tag="xTAO")

        # ---------- phase C: attention (per batch) ----------
        def normalize_group(b, qc, hpg, sgrp):
            """Reciprocal + normalize for pairs 4*hpg..4*hpg+3 at this qc
            (their 8 sum rows, collected in sgrp, are complete).
            sgrp row: (hp%4)*2 + (0 even head | 1 odd head)."""
            rg32 = grpp.tile([8, 512], f32, tag="rg32")
            rg = grpp.tile([8, 512], f16, tag="rg")
            nc.vector.reciprocal_approx_fast(out=rg32, in_=sgrp)
            nc.vector.tensor_copy(out=rg, in_=rg32)
            q0 = b * S + qc * 512
            for lp in range(4):
                p = 4 * hpg + lp                 # head pair == dt block
                bc = bcp.tile([128, 512], f16, tag="bc")
                for loc, p0 in ((2 * lp, 0), (2 * lp + 1, 64)):
                    r1 = rg[loc:loc + 1, :]
                    for ch in range(4):  # 4 parallel column chunks
                        rc = r1[:, ch * 128:(ch + 1) * 128]
                        rsrc = bass.AP(
                            tensor=rc.tensor, offset=rc.offset,
                            ap=[list(rc.ap[0]), [0, 64]]
                            + [list(a) for a in rc.ap[1:]])
                        nc.sync.dma_start(
                            out=bc[p0:p0 + 64, ch * 128:(ch + 1) * 128],
                            in_=rsrc)
                nc.vector.tensor_mul(
                    out=AO[:, p, q0:q0 + 512],
                    in0=AO[:, p, q0:q0 + 512], in1=bc)

        def attention_batch(b, interleave=None):
            for qc in range(2):
                nkt = (qc + 1) * 4
                q0 = b * S + qc * 512
                for hp in range(NDT):            # head pair == dt block
                    if hp % 4 == 0:
                        sgrp = grpp.tile([8, 512], f32, tag="sgrp")
                    hA, hB = 2 * hp, 2 * hp + 1
                    ps_oA = psO.tile([128, 512], f32, tag="psO")
                    ps_oB = psO.tile([128, 512], f32, tag="psO")

                    exts = []

                    def emit_scores(kt):
                        k0 = kt * 128
                        off = max(0, k0 - qc * 512)
                        kg = b * S + k0
                        exab = []
                        for po in (0, 64):   # even/odd head: PE row groups
                            ps_s = psS.tile([128, 512], f32, tag="psS")
                            nc.tensor.matmul(
                                ps_s[:, off:512],
                                KT[po:po + 64, hp, kg:kg + 128],
                                QT[po:po + 64, hp, q0 + off:q0 + 512],
                                start=True, stop=True)
                            ex = expp.tile([128, 512], f16, tag="exp")
                            nc.scalar.activation(
                                out=ex[:, off:512], in_=ps_s[:, off:512],
                                func=EXPF, scale=0.125)
                            if k0 >= qc * 512:  # diagonal: 0/1 triangle
                                nc.vector.tensor_mul(
                                    ex[:, off:off + 128],
                                    ex[:, off:off + 128], tri01)
                            exab.append(ex)
                        exts.append((exab[0], exab[1], off, kt))

                    def emit_av(i):
                        exA, exB, off, kt = exts[i]
                        mtv = b * (S // 128) + kt
                        for ex, ps_o, h in ((exA, ps_oA, hA),
                                            (exB, ps_oB, hB)):
                            nc.tensor.matmul(
                                ps_o[0:65, off:512],
                                V[:, mtv, h * 65:h * 65 + 65],
                                ex[:, off:512],
                                start=(kt == 0), stop=(kt == nkt - 1))

                    # software pipeline: scores run 4 blocks ahead of AV
                    for kt in range(nkt):
                        emit_scores(kt)
                        if kt >= 4:
                            emit_av(kt - 4)
                    for i in range(max(0, nkt - 4), nkt):
                        emit_av(i)

                    # evacuate unnormalized output + sum rows (DVE)
                    locA = (hp % 4) * 2
                    for loc, ps_o, po in ((locA, ps_oA, 0),
                                          (locA + 1, ps_oB, 64)):
                        srow = srp.tile([65, 512], f32, tag="srow")
                        nc.vector.tensor_copy(out=srow[64:65, :],
                                              in_=ps_o[64:65, :])
                        nc.sync.dma_start(out=sgrp[loc:loc + 1, :],
                                          in_=srow[64:65, :])
                        if po == 0:
                            nc.vector.tensor_copy(
                                out=AO[0:64, hp, q0:q0 + 512],
                                in_=ps_o[0:64, :])
                        else:
                            tmp = tmpp.tile([64, 512], f16, tag="tmp")
                            nc.vector.tensor_copy(out=tmp, in_=ps_o[0:64, :])
                            nc.sync.dma_start(
                                out=AO[64:128, hp, q0:q0 + 512], in_=tmp)
                    if hp % 4 == 3:
                        normalize_group(b, qc, hp // 4, sgrp)
                    if interleave is not None:
                        interleave(qc, hp)

        def out_proj_tile(mt, WoT):
            ys = ystage.tile([128, D], f32, tag="ys")
            for oc in range(NOC):
                pp = psAcc.tile([128, 512], f32, tag="psAcc")
                for dt_ in range(NDT):
                    nc.tensor.matmul(
                        pp,
                        AO[:, dt_, mt * 128:(mt + 1) * 128],
                        WoT[:, dt_, oc * 512:(oc + 1) * 512],
                        start=(dt_ == 0), stop=(dt_ == NDT - 1))
                nc.scalar.copy(out=ys[:, oc * 512:(oc + 1) * 512], in_=pp)
            nc.sync.dma_start(out=y_d[mt * 128:(mt + 1) * 128, :], in_=ys)

        attention_batch(0)
        WoT = wrot.tile([128, NDT, D], f16, tag="WT")
        load_transposed(WoT, wo_d, D)

        # attention b1 with phase-D m-tiles interleaved (they hide in the
        # ACT-bound stretches of attention):
        #   b1 qc=0 pairs: batch-0 m-tiles 0..7
        #   b1 qc=1 pairs: batch-1 qc=0 m-tiles 8..11
        def _ilv(qc, hp):
            if qc == 0:
                out_proj_tile(hp, WoT)
            elif hp % 2 == 1:
                out_proj_tile(8 + hp // 2, WoT)
        attention_batch(1, interleave=_ilv)
        for mt in range(12, 16):
            out_proj_tile(mt, WoT)

    nc.compile()
    return nc


def _tri01():
    # tri01[dk, dq] = 1 where k <= q (allowed), else 0
    return np.triu(np.ones((128, 128), np.float16))


def _get_nc():
    if "nc" not in _CACHE:
        _CACHE["nc"] = _build_nc()
    return _CACHE["nc"]


def kernel(x, Wq, bq, Wk, bk, Wv, bv, Wo, bo):
    from concourse.bass_utils import run_bass_kernel_spmd

    x = np.ascontiguousarray(np.asarray(x, dtype=np.float32))
    B = x.shape[0]
    assert x.shape == (B, S, D) and B == NCORES * BPC
    Wq = np.ascontiguousarray(np.asarray(Wq, dtype=np.float32))
    Wk = np.ascontiguousarray(np.asarray(Wk, dtype=np.float32))
    Wv = np.ascontiguousarray(np.asarray(Wv, dtype=np.float32))
    Wo = np.ascontiguousarray(np.asarray(Wo, dtype=np.float32))

    nc = _get_nc()
    shards = x.reshape(NCORES, M, D)
    tri = _tri01()
    in_maps = [
        {"x": shards[c], "Wq": Wq, "Wk": Wk, "Wv": Wv, "Wo": Wo, "tri01": tri}
        for c in range(NCORES)
    ]
    res = run_bass_kernel_spmd(nc, in_maps, core_ids=list(range(NCORES)))
    y = np.stack([res.results[c]["y"] for c in range(NCORES)])
    y = y.reshape(B, S, D)

    # exact host-side fold of bv and bo (bq/bk are zero by problem spec)
    bias = (np.asarray(bv, np.float32) @ np.asarray(Wo, np.float32).T
            + np.asarray(bo, np.float32))
    if np.any(bias):
        y = y + bias
    return y.astype(np.float32)


# revision 28
# speedup vs baseline: 1.1750x; 1.0032x over previous
"""Causal multi-head attention block (B=16, S=1024, d=1024, H=16) on 8 NeuronCores.

Strategy: data-parallel over batch (2 batches per core), no collectives.
Per-core kernel (fp16 matmuls, fp32 accumulation):
  phase A: x / W transposes via XBAR DMA-transpose (f32->f16 cast on ACT,
           then dma_start_transpose) -- keeps the PE free for matmuls
  phase B: QT = Wq @ xT, KT = Wk @ xT (transposed layout [d_out, m]),
           V  = x @ Wv.T (natural layout [m, d_out], packed in 65-wide
           per-head strips with a fused ones column)
  phase C: qc-major, head-PAIR iteration. The two heads of a pair have
           dh=64 contractions at partitions 0:64 / 64:128, so their score
           matmuls run CONCURRENTLY on disjoint PE row groups (auto
           tile_position from the lhsT base partition). exp((s+mask)/8) on
           ACT, causal mask via 0/1 triangle multiply on diagonal blocks,
           A@V accumulated per head (65-wide V strip with ones column ->
           sum row). Unnormalized outputs + sum rows evacuate on DVE;
           after every 4 pairs one batched DVE reciprocal_approx_fast
           computes 1/sums, chunked DMA broadcasts fan them out and one
           in-place DVE multiply per pair normalizes (ACT stays exp-only).
  phase D: y = attn_outT.T @ WoT (natural layout) -> DRAM; m-tiles are
           interleaved into attention emission so they hide in ACT-bound
           stretches.
Biases: bq/bk are zero by problem spec (ignored); bv/bo folded in exactly
on the host (y += bv @ Wo.T + bo).
"""

import numpy as np

_CACHE: dict = {}

S = 1024
D = 1024
H = 16
DH = 64
BPC = 2           # batches per core
M = BPC * S       # tokens per core
NCORES = 8


def _build_nc():
    import concourse.bass as bass  # noqa: F401
    import concourse.mybir as mybir
    import concourse.tile as tile
    from concourse import bacc
    from concourse.masks import make_identity
    from contextlib import ExitStack

    f32 = mybir.dt.float32
    f16 = mybir.dt.float16
    EXPF = mybir.ActivationFunctionType.Exp

    nc = bacc.Bacc("TRN2", target_bir_lowering=False, debug=False,
                   num_devices=NCORES)

    x_d = nc.dram_tensor("x", [M, D], f32, kind="ExternalInput")
    wq_d = nc.dram_tensor("Wq", [D, D], f32, kind="ExternalInput")
    wk_d = nc.dram_tensor("Wk", [D, D], f32, kind="ExternalInput")
    wv_d = nc.dram_tensor("Wv", [D, D], f32, kind="ExternalInput")
    wo_d = nc.dram_tensor("Wo", [D, D], f32, kind="ExternalInput")
    tri_d = nc.dram_tensor("tri01", [128, 128], f16, kind="ExternalInput")
    y_d = nc.dram_tensor("y", [M, D], f32, kind="ExternalOutput")

    NMT = M // 128        # 16 m-tiles
    NDT = D // 128        # 8 d-tiles
    NMC = M // 512        # 4 m-chunks
    NOC = D // 512        # 2 o-chunks

    with tile.TileContext(nc) as tc, ExitStack() as top:
        consts = top.enter_context(tc.tile_pool(name="consts", bufs=1))
        persist = top.enter_context(tc.tile_pool(name="persist", bufs=1))
        wrot = top.enter_context(tc.tile_pool(name="wrot", bufs=1))
        stage = top.enter_context(tc.tile_pool(name="stage", bufs=2))
        ystage = top.enter_context(tc.tile_pool(name="ystage", bufs=3))
        expp = top.enter_context(tc.tile_pool(name="expp", bufs=12))
        tmpp = top.enter_context(tc.tile_pool(name="tmpp", bufs=3))
        srp = top.enter_context(tc.tile_pool(name="srp", bufs=2))
        bcp = top.enter_context(tc.tile_pool(name="bcp", bufs=3))
        grpp = top.enter_context(tc.tile_pool(name="grpp", bufs=3))
        # PSUM: 3 + 3 + 2 = 8 banks
        psAcc = top.enter_context(tc.tile_pool(name="psAcc", bufs=3, space="PSUM"))
        psS = top.enter_context(tc.tile_pool(name="psS", bufs=3, space="PSUM"))
        psO = top.enter_context(tc.tile_pool(name="psO", bufs=2, space="PSUM"))

        ident = consts.tile([128, 128], f32, tag="ident")
        make_identity(nc, ident)
        tri01 = consts.tile([128, 128], f16, tag="tri")
        nc.sync.dma_start(out=tri01, in_=tri_d[:, :])

        # persistent activations (fp16)
        QT = persist.tile([128, NDT, M], f16, tag="QT")    # [o, m] transposed
        KT = persist.tile([128, NDT, M], f16, tag="KT")
        V = persist.tile([128, NMT, H * 65], f16, tag="V")  # [m, head strips]

        def load_transposed(dst, dram, ncols, row0=0, dve=False):
            """dst[:, i_tile, c*128:(c+1)*128] = dram[row0+c*128:...+128, :].T
            dst: [128, NDT, ncols] fp16; dram: [.., D] fp32. Evacuation on
            ACT by default, DVE when the ACT queue is exp-loaded."""
            for rt in range(ncols // 128):
                st = stage.tile([128, D], f32, tag="stage")
                r0 = row0 + rt * 128
                nc.sync.dma_start(out=st, in_=dram[r0:r0 + 128, :])
                for g in range(NDT // 4):
                    pt = psAcc.tile([128, 512], f32, tag="psAcc")
                    for c in range(4):
                        ib = g * 4 + c
                        nc.tensor.transpose(
                            pt[:, c * 128:(c + 1) * 128],
                            st[:, ib * 128:(ib + 1) * 128], ident)
                    cp = nc.vector.tensor_copy if dve else nc.scalar.copy
                    cp(out=dst[:, g * 4:g * 4 + 4, rt * 128:(rt + 1) * 128],
                       in_=pt.rearrange("p (a b) -> p a b", b=128))

        # ---------- phases A+B: projections ----------
        # Per-batch xT halves alias the per-batch AO halves: xT0's last
        # reader is the last batch-0 projection, AO0's first writer is the
        # batch-0 attention evacuation (WAR via slot reuse); same for b1.
        xTp = top.enter_context(tc.tile_pool(name="xTp", bufs=1))
        xT0 = xTp.tile([128, NDT, S], f16, tag="xa0")
        xT1 = xTp.tile([128, NDT, S], f16, tag="xa1")
        load_transposed(xT0, x_d, S)
        load_transposed(xT1, x_d, S, row0=S)
        xTb = (xT0, xT1)

        # V projection first (frees the Wv slot early), then Q, K
        WT = wrot.tile([128, NDT, D], f16, tag="WT")
        load_transposed(WT, wv_d, D)
        for mt in range(NMT):
            xTt, lmt = xTb[mt // 8], mt % 8
            v2 = V[:, mt, :].rearrange("p (a c) -> p a c", c=65)
            nc.gpsimd.memset(v2[:, :, 64], 1.0)
            for oc in range(NOC):
                pp = psAcc.tile([128, 512], f32, tag="psAcc")
                for it in range(NDT):
                    nc.tensor.matmul(
                        pp,
                        xTt[:, it, lmt * 128:(lmt + 1) * 128],
                        WT[:, it, oc * 512:(oc + 1) * 512],
                        start=(it == 0), stop=(it == NDT - 1))
                nc.scalar.copy(
                    out=v2[:, 8 * oc:8 * oc + 8, 0:64],
                    in_=pp.rearrange("p (a c) -> p a c", c=64))

        # Q and K projections -> transposed layout (batch-0 m-chunks first)
        for w_dram, dst in ((wq_d, QT), (wk_d, KT)):
            WT = wrot.tile([128, NDT, D], f16, tag="WT")
            load_transposed(WT, w_dram, D)
            for mc in range(NMC):
                xTt, lc = xTb[mc // 2], (mc % 2) * 512
                for ot in range(NDT):
                    pp = psAcc.tile([128, 512], f32, tag="psAcc")
                    for it in range(NDT):
                        nc.tensor.matmul(
                            pp,
                            WT[:, it, ot * 128:(ot + 1) * 128],
                            xTt[:, it, lc:lc + 512],
                            start=(it == 0), stop=(it == NDT - 1))
                    nc.scalar.copy(
                        out=dst[:, ot, mc * 512:(mc + 1) * 512], in_=pp)

        # attn out (transposed layout), reuses the xT slots per batch
        AO0 = xTp.tile([128, NDT, S], f16, tag="xa0")
        AO1 = xTp.tile([128, NDT, S], f16, tag="xa1")
        AOb = (AO0, AO1)

        # ---------- phase C: attention (per batch) ----------
        def normalize_group(b, qc, hpg, sgrp):
            """Reciprocal + normalize for pairs 4*hpg..4*hpg+3 at this qc
            (their 8 sum rows, collected in sgrp, are complete).
            sgrp row: (hp%4)*2 + (0 even head | 1 odd head)."""
            rg32 = grpp.tile([8, 512], f32, tag="rg32")
            rg = grpp.tile([8, 512], f16, tag="rg")
            nc.vector.reciprocal_approx_fast(out=rg32, in_=sgrp)
            nc.vector.tensor_copy(out=rg, in_=rg32)
            AO = AOb[b]
            lq0 = qc * 512
            for lp in range(4):
                p = 4 * hpg + lp                 # head pair == dt block
                bc = bcp.tile([128, 512], f16, tag="bc")
                for loc, p0 in ((2 * lp, 0), (2 * lp + 1, 64)):
                    r1 = rg[loc:loc + 1, :]
                    for ch in range(4):  # 4 parallel column chunks
                        rc = r1[:, ch * 128:(ch + 1) * 128]
                        rsrc = bass.AP(
                            tensor=rc.tensor, offset=rc.offset,
                            ap=[list(rc.ap[0]), [0, 64]]
                            + [list(a) for a in rc.ap[1:]])
                        nc.sync.dma_start(
                            out=bc[p0:p0 + 64, ch * 128:(ch + 1) * 128],
                            in_=rsrc)
                nc.vector.tensor_mul(
                    out=AO[:, p, lq0:lq0 + 512],
                    in0=AO[:, p, lq0:lq0 + 512], in1=bc)

        def attention_batch(b, interleave=None):
            AO = AOb[b]
            for qc in range(2):
                nkt = (qc + 1) * 4
                q0 = b * S + qc * 512
                lq0 = qc * 512
                for hp in range(NDT):            # head pair == dt block
                    if hp % 4 == 0:
                        sgrp = grpp.tile([8, 512], f32, tag="sgrp")
                    hA, hB = 2 * hp, 2 * hp + 1
                    ps_oA = psO.tile([128, 512], f32, tag="psO")
                    ps_oB = psO.tile([128, 512], f32, tag="psO")

                    exts = []

                    def emit_scores(kt):
                        k0 = kt * 128
                        off = max(0, k0 - qc * 512)
                        kg = b * S + k0
                        exab = []
                        for po in (0, 64):   # even/odd head: PE row groups
                            ps_s = psS.tile([128, 512], f32, tag="psS")
                            nc.tensor.matmul(
                                ps_s[:, off:512],
                                KT[po:po + 64, hp, kg:kg + 128],
                                QT[po:po + 64, hp, q0 + off:q0 + 512],
                                start=True, stop=True)
                            ex = expp.tile([128, 512], f16, tag="exp")
                            nc.scalar.activation(
                                out=ex[:, off:512], in_=ps_s[:, off:512],
                                func=EXPF, scale=0.125)
                            if k0 >= qc * 512:  # diagonal: 0/1 triangle
                                nc.vector.tensor_mul(
                                    ex[:, off:off + 128],
                                    ex[:, off:off + 128], tri01)
                            exab.append(ex)
                        exts.append((exab[0], exab[1], off, kt))

                    def emit_av(i):
                        exA, exB, off, kt = exts[i]
                        mtv = b * (S // 128) + kt
                        for ex, ps_o, h in ((exA, ps_oA, hA),
                                            (exB, ps_oB, hB)):
                            nc.tensor.matmul(
                                ps_o[0:65, off:512],
                                V[:, mtv, h * 65:h * 65 + 65],
                                ex[:, off:512],
                                start=(kt == 0), stop=(kt == nkt - 1))

                    # software pipeline: scores run 4 blocks ahead of AV
                    for kt in range(nkt):
                        emit_scores(kt)
                        if kt >= 4:
                            emit_av(kt - 4)
                    for i in range(max(0, nkt - 4), nkt):
                        emit_av(i)

                    # evacuate unnormalized output + sum rows (DVE)
                    locA = (hp % 4) * 2
                    for loc, ps_o, po in ((locA, ps_oA, 0),
                                          (locA + 1, ps_oB, 64)):
                        srow = srp.tile([65, 512], f32, tag="srow")
                        nc.vector.tensor_copy(out=srow[64:65, :],
                                              in_=ps_o[64:65, :])
                        nc.sync.dma_start(out=sgrp[loc:loc + 1, :],
                                          in_=srow[64:65, :])
                        if po == 0:
                            nc.vector.tensor_copy(
                                out=AO[0:64, hp, lq0:lq0 + 512],
                                in_=ps_o[0:64, :])
                        else:
                            tmp = tmpp.tile([64, 512], f16, tag="tmp")
                            nc.vector.tensor_copy(out=tmp, in_=ps_o[0:64, :])
                            nc.sync.dma_start(
                                out=AO[64:128, hp, lq0:lq0 + 512], in_=tmp)
                    if hp % 4 == 3:
                        normalize_group(b, qc, hp // 4, sgrp)
                    if interleave is not None:
                        interleave(qc, hp)

        def out_proj_tile(mt, WoT):
            AO, lmt = AOb[mt // 8], mt % 8
            ys = ystage.tile([128, D], f32, tag="ys")
            for oc in range(NOC):
                pp = psAcc.tile([128, 512], f32, tag="psAcc")
                for dt_ in range(NDT):
                    nc.tensor.matmul(
                        pp,
                        AO[:, dt_, lmt * 128:(lmt + 1) * 128],
                        WoT[:, dt_, oc * 512:(oc + 1) * 512],
                        start=(dt_ == 0), stop=(dt_ == NDT - 1))
                nc.vector.tensor_copy(out=ys[:, oc * 512:(oc + 1) * 512],
                                      in_=pp)
            nc.sync.dma_start(out=y_d[mt * 128:(mt + 1) * 128, :], in_=ys)

        attention_batch(0)
        WoT = wrot.tile([128, NDT, D], f16, tag="WT")
        load_transposed(WoT, wo_d, D, dve=True)

        # attention b1 with phase-D m-tiles interleaved (they hide in the
        # ACT-bound stretches of attention):
        #   b1 qc=0 pairs: batch-0 m-tiles 0..7
        #   b1 qc=1 pairs: batch-1 qc=0 m-tiles 8..11
        def _ilv(qc, hp):
            if qc == 0:
                out_proj_tile(hp, WoT)
            elif hp % 2 == 1:
                out_proj_tile(8 + hp // 2, WoT)
        attention_batch(1, interleave=_ilv)
        for mt in range(12, 16):
            out_proj_tile(mt, WoT)

    nc.compile()
    return nc


def _tri01():
    # tri01[dk, dq] = 1 where k <= q (allowed), else 0
    return np.triu(np.ones((128, 128), np.float16))


def _get_nc():
    if "nc" not in _CACHE:
        _CACHE["nc"] = _build_nc()
    return _CACHE["nc"]


def kernel(x, Wq, bq, Wk, bk, Wv, bv, Wo, bo):
    from concourse.bass_utils import run_bass_kernel_spmd

    x = np.ascontiguousarray(np.asarray(x, dtype=np.float32))
    B = x.shape[0]
    assert x.shape == (B, S, D) and B == NCORES * BPC
    Wq = np.ascontiguousarray(np.asarray(Wq, dtype=np.float32))
    Wk = np.ascontiguousarray(np.asarray(Wk, dtype=np.float32))
    Wv = np.ascontiguousarray(np.asarray(Wv, dtype=np.float32))
    Wo = np.ascontiguousarray(np.asarray(Wo, dtype=np.float32))

    nc = _get_nc()
    shards = x.reshape(NCORES, M, D)
    tri = _tri01()
    in_maps = [
        {"x": shards[c], "Wq": Wq, "Wk": Wk, "Wv": Wv, "Wo": Wo, "tri01": tri}
        for c in range(NCORES)
    ]
    res = run_bass_kernel_spmd(nc, in_maps, core_ids=list(range(NCORES)))
    y = np.stack([res.results[c]["y"] for c in range(NCORES)])
    y = y.reshape(B, S, D)

    # exact host-side fold of bv and bo (bq/bk are zero by problem spec)
    bias = (np.asarray(bv, np.float32) @ np.asarray(Wo, np.float32).T
            + np.asarray(bo, np.float32))
    if np.any(bias):
        y = y + bias
    return y.astype(np.float32)
